# revision 9
# baseline (speedup 1.0000x reference)
"""Trainium2 Bass kernel for nn_MultiHeadedAttention (B=4, S=2048, D=1024, H=16).

Sharding: 8 cores = 4 batches x 2 head-groups. Each core computes, for its
batch b and its 8 heads (a 512-wide column slice of Wq/Wk/Wv and row slice
of Wo):
  - QKV projections (with on-PE input transposes),
  - causal+pad masked softmax probabilities (full [8, S, S] written out),
  - attention output and its partial output projection [S, D].
Host sums the two partial hidden projections per batch (+bo) and concatenates
probability shards.

All matmuls run in float32r (TF32-like, full PE rate, ~1.5e-4 rel err).
reference semantics reproduced exactly:
  corr = (q_h k_h^T)/8 ; masked (pad row OR k>q) -> -1e9 -> softmax
  pad rows therefore become uniform 1/S; masked entries exact 0.
"""

import numpy as np
from contextlib import ExitStack

import concourse.bass as bass
import concourse.mybir as mybir
import concourse.tile as tile
from concourse import bacc
from concourse.bass_utils import run_bass_kernel_spmd

f32 = mybir.dt.float32
f32r = mybir.dt.float32r
AF = mybir.ActivationFunctionType
ALU = mybir.AluOpType

B, S_FULL, D, HEADS, DH = 4, 2048, 1024, 16, 64
N_CORES = 8
GROUPS = 2                      # head-groups (tensor-parallel dimension)
HPC_FULL = HEADS // GROUPS      # heads per core


def _bcast(col: bass.AP, w: int) -> bass.AP:
    """[P,1] column slice -> [P,w] stride-0 broadcast AP (compute engines)."""
    return bass.AP(tensor=col.tensor, offset=col.offset, ap=[col.ap[0], [0, w]])


def build_nc(S: int = S_FULL, HPC: int = HPC_FULL):
    """Build the per-core SPMD program. S must be a multiple of 512."""
    NQT = S // 128          # 128-row q subtiles
    SUP = NQT // 4          # 512-row q super tiles
    PAIRS = HPC // 2        # head pairs
    DCOL = HPC * DH         # this core's D-slice width (512 full size)
    KT = D // 128           # contraction tiles for projections

    nc = bacc.Bacc("TRN2", target_bir_lowering=False, debug=False,
                   num_devices=N_CORES)

    # ---- DRAM I/O ----
    xq = nc.dram_tensor("xq", [S, D], f32, kind="ExternalInput")
    xk = nc.dram_tensor("xk", [S, D], f32, kind="ExternalInput")
    xv = nc.dram_tensor("xv", [S, D], f32, kind="ExternalInput")
    wq = nc.dram_tensor("wq", [D, DCOL], f32, kind="ExternalInput")
    wk = nc.dram_tensor("wk", [D, DCOL], f32, kind="ExternalInput")
    wv = nc.dram_tensor("wv", [D, DCOL], f32, kind="ExternalInput")
    wo = nc.dram_tensor("wo", [DCOL, D], f32, kind="ExternalInput")
    b2q = nc.dram_tensor("b2q", [128, PAIRS], f32, kind="ExternalInput")
    b2k = nc.dram_tensor("b2k", [128, PAIRS], f32, kind="ExternalInput")
    b2v = nc.dram_tensor("b2v", [128, PAIRS], f32, kind="ExternalInput")
    keep = nc.dram_tensor("keep", [128, NQT], f32, kind="ExternalInput")
    padh = nc.dram_tensor("padh", [128, NQT], f32, kind="ExternalInput")
    padrow = nc.dram_tensor("padrow", [1, S], f32, kind="ExternalInput")
    tri = nc.dram_tensor("tri", [128, 128], f32, kind="ExternalInput")
    ones_in = nc.dram_tensor("ones_in", [128, 128], f32, kind="ExternalInput")
    eye = nc.dram_tensor("eye", [128, 128], f32, kind="ExternalInput")
    prob = nc.dram_tensor("prob", [HPC, S, S], f32, kind="ExternalOutput")
    hid = nc.dram_tensor("hid", [S, D], f32, kind="ExternalOutput")

    with tile.TileContext(nc) as tc, ExitStack() as ctx0:
        pers = ctx0.enter_context(tc.tile_pool(name="pers", bufs=1))

        qhT = pers.tile([128, PAIRS, S], f32r)   # [2-head dh, pair, s]
        khT = pers.tile([128, PAIRS, S], f32r)
        vhn = pers.tile([128, S // 128, DCOL], f32r)  # [k%128, ktile, d-slice]
        wo_sb = pers.tile([128, DCOL // 128, D], f32r)
        tri_sb = pers.tile([128, 128], f32)
        eye_f = pers.tile([128, 128], f32)
        eye_r = pers.tile([128, 128], f32r)
        keep_sb = pers.tile([128, NQT], f32)
        padh_sb = pers.tile([128, NQT], f32)
        padrow_sb = pers.tile([1, S], f32r)
        ones_row = pers.tile([1, 128], f32r)
        ones_col2 = pers.tile([128, 2], f32r)
        b2_sb = {}
        for n in ("q", "k", "v"):
            b2_tile = pers.tile([128, PAIRS], f32, tag=f"b2{n}")
            b2_sb[n] = b2_tile
        if SUP > 1:
            sfxT = pers.tile([1, PAIRS, SUP - 1, 128], f32r)

        nc.sync.dma_start(out=tri_sb, in_=tri[:])
        nc.sync.dma_start(out=eye_f, in_=eye[:])
        nc.gpsimd.dma_start(out=eye_r, in_=eye[:])
        nc.sync.dma_start(out=keep_sb, in_=keep[:])
        nc.sync.dma_start(out=padh_sb, in_=padh[:])
        nc.gpsimd.dma_start(out=padrow_sb, in_=padrow[:])
        nc.sync.dma_start(out=b2_sb["q"], in_=b2q[:])
        nc.sync.dma_start(out=b2_sb["k"], in_=b2k[:])
        nc.sync.dma_start(out=b2_sb["v"], in_=b2v[:])
        nc.gpsimd.dma_start(out=ones_row, in_=ones_in[0:1, :])
        nc.gpsimd.dma_start(out=ones_col2, in_=ones_in[:, 0:2])
        for dk in range(DCOL // 128):
            nc.gpsimd.dma_start(out=wo_sb[:, dk, :],
                                in_=wo[128 * dk:128 * (dk + 1), :])

        # ================= Phase A: transposes + projections =================
        with ExitStack() as ctxA:
            wpool = ctxA.enter_context(tc.tile_pool(name="wA", bufs=2))
            natp = ctxA.enter_context(tc.tile_pool(name="nat", bufs=6))
            xtp = ctxA.enter_context(tc.tile_pool(name="xt", bufs=3))
            vtp = ctxA.enter_context(tc.tile_pool(name="vt", bufs=2))
            psA = ctxA.enter_context(tc.tile_pool(name="psA", bufs=PAIRS,
                                                  space="PSUM"))
            pstp = ctxA.enter_context(tc.tile_pool(name="pstp", bufs=2,
                                                   space="PSUM"))
            pst2 = ctxA.enter_context(tc.tile_pool(name="pst2", bufs=2,
                                                   space="PSUM"))

            for name, x_t, w_t, dest in (("q", xq, wq, qhT), ("k", xk, wk, khT),
                                         ("v", xv, wv, None)):
                w_sb = wpool.tile([128, KT, DCOL], f32r, tag="w")
                for d in range(KT):
                    nc.gpsimd.dma_start(out=w_sb[:, d, :],
                                        in_=w_t[128 * d:128 * (d + 1), :])
                for n in range(S // 512):
                    nats = []
                    for ss in range(4):
                        nat = natp.tile([128, D], f32, tag="nat")
                        r0 = 512 * n + 128 * ss
                        nc.sync.dma_start(out=nat, in_=x_t[r0:r0 + 128, :])
                        nats.append(nat)
                    proj_ps = [psA.tile([128, 512], f32, tag="proj",
                                        name=f"proj{n}_{p}")
                               for p in range(PAIRS)]
                    for d in range(KT):
                        tp = pstp.tile([128, 512], f32, tag="tp")
                        for ss in range(4):
                            nc.tensor.transpose(
                                tp[:, 128 * ss:128 * (ss + 1)],
                                nats[ss][:, 128 * d:128 * (d + 1)], eye_f)
                        xt = xtp.tile([128, 512], f32r, tag="xt")
                        nc.scalar.copy(xt, tp)
                        for p in range(PAIRS):
                            nc.tensor.matmul(
                                proj_ps[p],
                                w_sb[:, d, 128 * p:128 * (p + 1)], xt,
                                start=(d == 0), stop=(d == KT - 1))
                    for p in range(PAIRS):
                        if dest is not None:
                            nc.scalar.activation(
                                dest[:, p, 512 * n:512 * (n + 1)], proj_ps[p],
                                AF.Identity, bias=b2_sb[name][:, p:p + 1])
                        else:
                            vt = vtp.tile([128, 512], f32r, tag="vt")
                            nc.scalar.activation(
                                vt, proj_ps[p], AF.Identity,
                                bias=b2_sb["v"][:, p:p + 1])
                            for ss in range(4):
                                t2 = pst2.tile([128, 128], f32r, tag="t2")
                                nc.tensor.transpose(
                                    t2, vt[:, 128 * ss:128 * (ss + 1)], eye_r)
                                nc.scalar.copy(
                                    vhn[:, 4 * n + ss,
                                        128 * p:128 * (p + 1)], t2)

            # suffix sums of V per pair (for pad-row tail correction)
            if SUP > 1:
                smt = ctxA.enter_context(tc.tile_pool(name="smt", bufs=2))
                NT = S // 128
                for p in range(PAIRS):
                    ts_ps = pstp.tile([128, 2 * NT], f32, tag="tp")
                    for t in range(NT):
                        nc.tensor.matmul(ts_ps[:, 2 * t:2 * t + 2],
                                         vhn[:, t, 128 * p:128 * (p + 1)],
                                         ones_col2, start=True, stop=True)
                    ts_sb = smt.tile([128, NT, 2], f32, tag="tssb")
                    nc.scalar.copy(ts_sb, ts_ps[:].rearrange(
                        "q (t two) -> q t two", two=2))
                    vs = smt.tile([128, SUP], f32, tag="vs")
                    nc.vector.memset(vs, 0.0)
                    for I in range(SUP - 1):
                        nc.vector.reduce_sum(vs[:, I:I + 1],
                                             ts_sb[:, 4 * (I + 1):NT, 0],
                                             axis=mybir.AxisListType.X)
                    tr_ps = pst2.tile([SUP, 128], f32, tag="t2")
                    nc.tensor.transpose(tr_ps, vs, eye_f)
                    sfx_sb = smt.tile([SUP, 128], f32r, tag="sfxsb")
                    nc.scalar.copy(sfx_sb, tr_ps)
                    for I in range(SUP - 1):
                        nc.sync.dma_start(out=sfxT[0:1, p, I, :],
                                          in_=sfx_sb[I:I + 1, :])

        # ======================= Phase B: attention ==========================
        with ExitStack() as ctxB:
            expool = ctxB.enter_context(tc.tile_pool(name="exp", bufs=3))
            ppool = ctxB.enter_context(tc.tile_pool(name="p", bufs=4))
            pfpool = ctxB.enter_context(tc.tile_pool(name="pf", bufs=2))
            ptpool = ctxB.enter_context(tc.tile_pool(name="pt", bufs=3))
            osup = ctxB.enter_context(tc.tile_pool(name="osup", bufs=1))
            hidp = ctxB.enter_context(tc.tile_pool(name="hid", bufs=1))
            smallp = ctxB.enter_context(tc.tile_pool(name="small", bufs=10))
            otpool = ctxB.enter_context(tc.tile_pool(name="ot", bufs=2))
            psS = ctxB.enter_context(tc.tile_pool(name="psS", bufs=2,
                                                  space="PSUM"))
            psT = ctxB.enter_context(tc.tile_pool(name="psT", bufs=2,
                                                  space="PSUM"))
            psO = ctxB.enter_context(tc.tile_pool(name="psO", bufs=1,
                                                  space="PSUM"))
            psOP = ctxB.enter_context(tc.tile_pool(name="psOP", bufs=1,
                                                   space="PSUM"))

            for I in range(SUP):
                # pad-row fill of the not-computed (k beyond causal) region,
                # broadcast to all heads in one DMA per subtile.
                for m in range(4):
                    i = 4 * I + m
                    Ni = 128 * (i + 1)
                    Wd = S - Ni
                    if Wd == 0:
                        continue
                    Wp = min(Wd, 1024)
                    pf = pfpool.tile([128, Wp], f32, tag="pf")
                    nc.vector.tensor_copy(pf, _bcast(padh_sb[:, i:i + 1], Wp))
                    c0 = 0
                    while c0 < Wd:
                        cw = min(Wp, Wd - c0)
                        dst = bass.AP(
                            tensor=prob, offset=128 * i * S + Ni + c0,
                            ap=[[S, 128], [S * S, HPC], [1, cw]])
                        pfap = pf[:]
                        src = bass.AP(
                            tensor=pfap.tensor, offset=pfap.offset,
                            ap=[pfap.ap[0], [0, HPC], [1, cw]])
                        nc.sync.dma_start(out=dst, in_=src)
                        c0 += cw

                o_sup = osup.tile([128, PAIRS, 512], f32r, tag="osup")
                for h in range(HPC):
                    pr, ro = h // 2, 64 * (h % 2)
                    p_list = []
                    for m in range(4):
                        i = 4 * I + m
                        Ni = 128 * (i + 1)
                        p_t = ppool.tile([128, S], f32r, tag="p")
                        exps = []
                        sums = []
                        base = 0
                        while base < Ni:
                            wdt = min(1024, Ni - base)
                            sps = psS.tile([128, wdt], f32, tag="s")
                            for c0 in range(0, wdt, 512):
                                cw = min(512, wdt - c0)
                                nc.tensor.matmul(
                                    sps[:, c0:c0 + cw],
                                    qhT[ro:ro + 64, pr, 128 * i:128 * (i + 1)],
                                    khT[ro:ro + 64, pr,
                                        base + c0:base + c0 + cw],
                                    start=True, stop=True)
                            if base + wdt == Ni:
                                nc.vector.tensor_add(
                                    sps[:, wdt - 128:wdt],
                                    sps[:, wdt - 128:wdt], tri_sb)
                            ex = expool.tile([128, wdt], f32, tag="e")
                            sm = smallp.tile([128, 1], f32, tag="sm")
                            nc.scalar.activation(ex, sps, AF.Exp, scale=0.125,
                                                 accum_out=sm)
                            exps.append((ex, base, wdt))
                            sums.append(sm)
                            base += wdt
                        if len(sums) == 2:
                            st = smallp.tile([128, 1], f32, tag="st")
                            nc.vector.tensor_add(st, sums[0], sums[1])
                        else:
                            st = sums[0]
                        rec = smallp.tile([128, 1], f32, tag="rec")
                        nc.vector.reciprocal(rec, st)
                        scl = smallp.tile([128, 1], f32, tag="scl")
                        nc.vector.tensor_mul(scl, rec, keep_sb[:, i:i + 1])
                        for (ex, b0, wdt) in exps:
                            nc.vector.tensor_scalar(
                                out=p_t[:, b0:b0 + wdt], in0=ex,
                                scalar1=scl, scalar2=padh_sb[:, i:i + 1],
                                op0=ALU.mult, op1=ALU.add)
                        nc.sync.dma_start(
                            out=prob[h, 128 * i:128 * (i + 1), 0:Ni],
                            in_=p_t[:, 0:Ni].bitcast(f32))
                        p_list.append(p_t)

                    # transpose P, PV matmuls
                    o_ps = psO.tile([64, 512], f32, tag="o")
                    last_j = 4 * I + 3
                    for j in range(last_j + 1):
                        tps = psT.tile([128, 512], f32r, tag="t")
                        for m in range(4):
                            i = 4 * I + m
                            if j <= i:
                                nc.tensor.transpose(
                                    tps[:, 128 * m:128 * (m + 1)],
                                    p_list[m][:, 128 * j:128 * (j + 1)],
                                    eye_r)
                            else:
                                nc.tensor.matmul(
                                    tps[:, 128 * m:128 * (m + 1)].bitcast(f32),
                                    ones_row.bitcast(f32),
                                    padrow_sb[0:1, 128 * i:128 * (i + 1)].bitcast(f32),
                                    start=True, stop=True)
                        pt = ptpool.tile([128, 512], f32r, tag="pt")
                        if j % 2 == 0:
                            nc.vector.tensor_copy(pt, tps)
                        else:
                            nc.scalar.copy(pt, tps)
                        nc.tensor.matmul(
                            o_ps, vhn[:, j, 128 * pr + ro:128 * pr + ro + 64],
                            pt, start=(j == 0),
                            stop=(j == last_j and I == SUP - 1))
                    if I < SUP - 1:
                        nc.tensor.matmul(
                            o_ps, sfxT[0:1, pr, I, ro:ro + 64],
                            padrow_sb[0:1, 512 * I:512 * (I + 1)],
                            start=False, stop=True)
                    # place O^T block; odd heads need a cross-partition DMA
                    if h % 2 == 0:
                        nc.scalar.copy(o_sup[0:64, pr, :], o_ps)
                    else:
                        otmp = otpool.tile([64, 512], f32r, tag="otmp")
                        nc.scalar.copy(otmp, o_ps)
                        nc.sync.dma_start(out=o_sup[64:128, pr, :], in_=otmp)

                # output projection for this q super-tile
                for qm in range(4):
                    hid_t = hidp.tile([128, D], f32, tag="hid")
                    for n2 in range(D // 512):
                        op_ps = psOP.tile([128, 512], f32, tag="op")
                        for dk in range(DCOL // 128):
                            nc.tensor.matmul(
                                op_ps,
                                o_sup[:, dk, 128 * qm:128 * (qm + 1)],
                                wo_sb[:, dk, 512 * n2:512 * (n2 + 1)],
                                start=(dk == 0), stop=(dk == DCOL // 128 - 1))
                        nc.scalar.copy(hid_t[:, 512 * n2:512 * (n2 + 1)],
                                       op_ps)
                    r0 = 512 * I + 128 * qm
                    nc.sync.dma_start(out=hid[r0:r0 + 128, :], in_=hid_t)

    nc.compile()
    return nc


# ---------------------------------------------------------------------------
# Host side
# ---------------------------------------------------------------------------

_NC_CACHE = {}


def _get_nc(S=S_FULL, HPC=HPC_FULL):
    key = (S, HPC)
    if key not in _NC_CACHE:
        _NC_CACHE[key] = build_nc(S, HPC)
    return _NC_CACHE[key]


def make_in_maps(q, k, v, mask, Wq, bq, Wk, bk, Wv, bv, Wo, bo,
                 S=S_FULL, HPC=HPC_FULL):
    """Build per-core input maps. q/k/v: [B,S,D]; mask: [B,S] bool."""
    NQT = S // 128
    DCOL = HPC * DH
    tri = np.triu(np.full((128, 128), -1e30, np.float32), 1)
    eye = np.eye(128, dtype=np.float32)
    in_maps = []
    for c in range(N_CORES):
        b, g = c // GROUPS, c % GROUPS
        sl = slice(DCOL * g, DCOL * (g + 1))
        keep = 1.0 - mask[b].astype(np.float32)
        padv = mask[b].astype(np.float32) / np.float32(S)
        in_maps.append({
            "xq": np.ascontiguousarray(q[b]),
            "xk": np.ascontiguousarray(k[b]),
            "xv": np.ascontiguousarray(v[b]),
            "wq": np.ascontiguousarray(Wq[:, sl]),
            "wk": np.ascontiguousarray(Wk[:, sl]),
            "wv": np.ascontiguousarray(Wv[:, sl]),
            "wo": np.ascontiguousarray(Wo[sl, :]),
            "b2q": np.ascontiguousarray(bq[sl].reshape(-1, 128).T),
            "b2k": np.ascontiguousarray(bk[sl].reshape(-1, 128).T),
            "b2v": np.ascontiguousarray(bv[sl].reshape(-1, 128).T),
            "keep": np.ascontiguousarray(keep.reshape(NQT, 128).T),
            "padh": np.ascontiguousarray(padv.reshape(NQT, 128).T),
            "padrow": np.ascontiguousarray(padv[None, :]),
            "tri": tri,
            "eye": eye,
            "ones_in": np.ones((128, 128), np.float32),
        })
    return in_maps


def assemble(results, bo, S=S_FULL, HPC=HPC_FULL):
    prob = np.empty((B, HEADS, S, S), np.float32)
    hidden = np.empty((B, S, D), np.float32)
    for c in range(N_CORES):
        b, g = c // GROUPS, c % GROUPS
        prob[b, HPC * g:HPC * (g + 1)] = results[c]["prob"]
    for b in range(B):
        hidden[b] = results[GROUPS * b]["hid"]
        for g in range(1, GROUPS):
            hidden[b] += results[GROUPS * b + g]["hid"]
        hidden[b] += bo
    return hidden, prob


def kernel(q, k, v, mask, Wq, bq, Wk, bk, Wv, bv, Wo, bo, reverse):
    q = np.asarray(q, np.float32)
    k = np.asarray(k, np.float32)
    v = np.asarray(v, np.float32)
    mask = np.asarray(mask, bool)
    Wq, bq = np.asarray(Wq, np.float32), np.asarray(bq, np.float32)
    Wk, bk = np.asarray(Wk, np.float32), np.asarray(bk, np.float32)
    Wv, bv = np.asarray(Wv, np.float32), np.asarray(bv, np.float32)
    Wo, bo = np.asarray(Wo, np.float32), np.asarray(bo, np.float32)
    rev = int(np.asarray(reverse))

    if rev:
        q, k, v, mask = (q[:, ::-1], k[:, ::-1], v[:, ::-1], mask[:, ::-1])

    nc = _get_nc()
    in_maps = make_in_maps(q, k, v, mask, Wq, bq, Wk, bk, Wv, bv, Wo, bo)
    res = run_bass_kernel_spmd(nc, in_maps, core_ids=list(range(N_CORES)))
    hidden, prob = assemble(res.results, bo)

    if rev:
        hidden = np.ascontiguousarray(hidden[:, ::-1])
        prob = np.ascontiguousarray(prob[:, :, ::-1, ::-1])
    return hidden, prob


# revision 15
# speedup vs baseline: 83.0663x; 83.0663x over previous
"""Trainium2 Bass kernel for nn_MultiHeadedAttention (B=4, S=2048, D=1024, H=16).

Sharding: 8 cores = 4 batches x 2 head-groups. Each core computes, for its
batch b and its 8 heads (a 512-wide column slice of Wq/Wk/Wv and row slice
of Wo):
  - QKV projections (with on-PE input transposes),
  - causal+pad masked softmax probabilities (full [8, S, S] written out),
  - attention output and its partial output projection [S, D].
Host sums the two partial hidden projections per batch (+bo) and concatenates
probability shards.

All matmuls run in float32r (TF32-like, full PE rate, ~1.5e-4 rel err).
reference semantics reproduced exactly:
  corr = (q_h k_h^T)/8 ; masked (pad row OR k>q) -> -1e9 -> softmax
  pad rows therefore become uniform 1/S; masked entries exact 0.
"""

import numpy as np
from contextlib import ExitStack

import concourse.bass as bass
import concourse.mybir as mybir
import concourse.tile as tile
from concourse import bacc
from concourse.bass_utils import run_bass_kernel_spmd

f32 = mybir.dt.float32
f32r = mybir.dt.float32r
AF = mybir.ActivationFunctionType
ALU = mybir.AluOpType

B, S_FULL, D, HEADS, DH = 4, 2048, 1024, 16, 64
N_CORES = 8
GROUPS = 2                      # head-groups (tensor-parallel dimension)
HPC_FULL = HEADS // GROUPS      # heads per core

BLEND_ON_POOL = True            # run softmax blend on GpSimd (frees DVE)


def _bcast(col: bass.AP, w: int) -> bass.AP:
    """[P,1] column slice -> [P,w] stride-0 broadcast AP (compute engines)."""
    return bass.AP(tensor=col.tensor, offset=col.offset, ap=[col.ap[0], [0, w]])


def build_nc(S: int = S_FULL, HPC: int = HPC_FULL, reps: int = 1):
    """Build the per-core SPMD program. S must be a multiple of 512."""
    NQT = S // 128
    PAIRS = HPC // 2
    DCOL = HPC * DH

    nc = bacc.Bacc("TRN2", target_bir_lowering=False, debug=False,
                   num_devices=N_CORES)

    nc.dram_tensor("xq", [S, D], f32, kind="ExternalInput")
    nc.dram_tensor("xk", [S, D], f32, kind="ExternalInput")
    nc.dram_tensor("xv", [S, D], f32, kind="ExternalInput")
    nc.dram_tensor("wq", [D, DCOL], f32, kind="ExternalInput")
    nc.dram_tensor("wk", [D, DCOL], f32, kind="ExternalInput")
    nc.dram_tensor("wv", [D, DCOL], f32, kind="ExternalInput")
    nc.dram_tensor("wo", [DCOL, D], f32, kind="ExternalInput")
    nc.dram_tensor("b2q", [128, PAIRS], f32, kind="ExternalInput")
    nc.dram_tensor("b2k", [128, PAIRS], f32, kind="ExternalInput")
    nc.dram_tensor("b2v", [128, PAIRS], f32, kind="ExternalInput")
    nc.dram_tensor("keep", [128, NQT], f32, kind="ExternalInput")
    nc.dram_tensor("padh", [128, NQT], f32, kind="ExternalInput")
    nc.dram_tensor("padrow", [1, S], f32, kind="ExternalInput")
    nc.dram_tensor("tri", [128, 128], f32, kind="ExternalInput")
    nc.dram_tensor("eye", [128, 128], f32, kind="ExternalInput")
    nc.dram_tensor("ones_in", [128, 128], f32, kind="ExternalInput")
    nc.dram_tensor("prob", [HPC, S, S], f32, kind="ExternalOutput")
    nc.dram_tensor("hid", [S, D], f32, kind="ExternalOutput")

    with tile.TileContext(nc) as tc:
        if reps > 1:
            with tc.For_i(0, reps, 1):
                _emit_body(nc, tc, S, HPC)
        else:
            _emit_body(nc, tc, S, HPC)

    nc.compile()
    return nc


def _emit_body(nc, tc, S, HPC):
    NQT = S // 128          # 128-row q subtiles
    SUP = NQT // 4          # 512-row q super tiles
    NCH = S // 512          # 512-wide sequence chunks
    PAIRS = HPC // 2
    DCOL = HPC * DH
    KT = D // 128

    f = nc.m.functions[0]
    names = {a.memorylocations[0].name: a for a in f.allocations
             if hasattr(a, "memorylocations") and a.memorylocations}

    def T(nm):
        a = names[nm]
        return bass.DRamTensorHandle(nm, list(a.tensor_shape), a.dtype)

    xq, xk, xv = T("xq"), T("xk"), T("xv")
    wq, wk, wv, wo = T("wq"), T("wk"), T("wv"), T("wo")
    b2q, b2k, b2v = T("b2q"), T("b2k"), T("b2v")
    keep, padh, padrow = T("keep"), T("padh"), T("padrow")
    tri, eye, ones_in = T("tri"), T("eye"), T("ones_in")
    prob, hid = T("prob"), T("hid")

    blend_eng = nc.gpsimd if BLEND_ON_POOL else nc.vector

    with ExitStack() as ctx0:
        pers = ctx0.enter_context(tc.tile_pool(name="pers", bufs=1))

        # per-512-chunk tiles so phase B can start before phase A finishes
        qhT_c = [pers.tile([128, PAIRS, 512], f32r, name=f"qhT{n}")
                 for n in range(NCH)]
        khT_c = [pers.tile([128, PAIRS, 512], f32r, name=f"khT{n}")
                 for n in range(NCH)]
        vhn_c = [pers.tile([128, 4, DCOL], f32r, name=f"vhn{n}")
                 for n in range(NCH)]
        wo_sb = pers.tile([128, DCOL // 128, D], f32r)
        tri_sb = pers.tile([128, 128], f32)
        eye_f = pers.tile([128, 128], f32)
        eye_r = pers.tile([128, 128], f32r)
        keep_sb = pers.tile([128, NQT], f32)
        padh_sb = pers.tile([128, NQT], f32)
        padrow_sb = pers.tile([1, S], f32r)
        ones_row = pers.tile([1, 128], f32r)
        ones_col2 = pers.tile([128, 2], f32r)
        b2_sb = {}
        for n in ("q", "k", "v"):
            b2_tile = pers.tile([128, PAIRS], f32, tag=f"b2{n}")
            b2_sb[n] = b2_tile
        if SUP > 1:
            sfxT = pers.tile([1, PAIRS, SUP - 1, 128], f32r)

        nc.sync.dma_start(out=tri_sb, in_=tri[:])
        nc.sync.dma_start(out=eye_f, in_=eye[:])
        nc.gpsimd.dma_start(out=eye_r, in_=eye[:])
        nc.sync.dma_start(out=keep_sb, in_=keep[:])
        nc.sync.dma_start(out=padh_sb, in_=padh[:])
        nc.gpsimd.dma_start(out=padrow_sb, in_=padrow[:])
        nc.sync.dma_start(out=b2_sb["q"], in_=b2q[:])
        nc.sync.dma_start(out=b2_sb["k"], in_=b2k[:])
        nc.sync.dma_start(out=b2_sb["v"], in_=b2v[:])
        nc.gpsimd.dma_start(out=ones_row, in_=ones_in[0:1, :])
        nc.gpsimd.dma_start(out=ones_col2, in_=ones_in[:, 0:2])
        for dk in range(DCOL // 128):
            nc.gpsimd.dma_start(out=wo_sb[:, dk, :],
                                in_=wo[128 * dk:128 * (dk + 1), :])

        # ================= Phase A: transposes + projections =================
        with ExitStack() as ctxA:
            wpool = ctxA.enter_context(tc.tile_pool(name="wA", bufs=2))
            natp = ctxA.enter_context(tc.tile_pool(name="nat", bufs=6))
            xtp = ctxA.enter_context(tc.tile_pool(name="xt", bufs=3))
            vtp = ctxA.enter_context(tc.tile_pool(name="vt", bufs=2))
            psA = ctxA.enter_context(tc.tile_pool(name="psA", bufs=5,
                                                  space="PSUM"))
            pstp = ctxA.enter_context(tc.tile_pool(name="pstp", bufs=2,
                                                   space="PSUM"))
            pst2 = ctxA.enter_context(tc.tile_pool(name="pst2", bufs=1,
                                                   space="PSUM"))

            for name, x_t, w_t, dest in (("q", xq, wq, qhT_c),
                                         ("k", xk, wk, khT_c),
                                         ("v", xv, wv, None)):
                w_sb = wpool.tile([128, KT, DCOL], f32r, tag="w")
                for d in range(KT):
                    nc.gpsimd.dma_start(out=w_sb[:, d, :],
                                        in_=w_t[128 * d:128 * (d + 1), :])
                for n in range(NCH):
                    nats = []
                    for ss in range(4):
                        nat = natp.tile([128, D], f32, tag="nat")
                        r0 = 512 * n + 128 * ss
                        nc.sync.dma_start(out=nat, in_=x_t[r0:r0 + 128, :])
                        nats.append(nat)
                    proj_ps = [psA.tile([128, 512], f32, tag="proj",
                                        name=f"proj{n}_{p}")
                               for p in range(PAIRS)]
                    for d in range(KT):
                        tp = pstp.tile([128, 512], f32, tag="tp")
                        for ss in range(4):
                            nc.tensor.transpose(
                                tp[:, 128 * ss:128 * (ss + 1)],
                                nats[ss][:, 128 * d:128 * (d + 1)], eye_f)
                        xt = xtp.tile([128, 512], f32r, tag="xt")
                        nc.scalar.copy(xt, tp)
                        for p in range(PAIRS):
                            nc.tensor.matmul(
                                proj_ps[p],
                                w_sb[:, d, 128 * p:128 * (p + 1)], xt,
                                start=(d == 0), stop=(d == KT - 1))
                    for p in range(PAIRS):
                        if dest is not None:
                            nc.scalar.activation(
                                dest[n][:, p, :], proj_ps[p],
                                AF.Identity, bias=b2_sb[name][:, p:p + 1])
                        else:
                            vt = vtp.tile([128, 512], f32r, tag="vt")
                            nc.scalar.activation(
                                vt, proj_ps[p], AF.Identity,
                                bias=b2_sb["v"][:, p:p + 1])
                            for ss in range(4):
                                t2 = pst2.tile([128, 128], f32r, tag="t2")
                                nc.tensor.transpose(
                                    t2, vt[:, 128 * ss:128 * (ss + 1)], eye_r)
                                nc.scalar.copy(
                                    vhn_c[n][:, ss, 128 * p:128 * (p + 1)], t2)

            # suffix sums of V per pair (for pad-row tail correction)
            if SUP > 1:
                smt = ctxA.enter_context(tc.tile_pool(name="smt", bufs=2))
                for p in range(PAIRS):
                    ts_ps = pstp.tile([128, 2 * NQT], f32, tag="tp")
                    for t in range(NQT):
                        nc.tensor.matmul(ts_ps[:, 2 * t:2 * t + 2],
                                         vhn_c[t // 4][:, t % 4,
                                                       128 * p:128 * (p + 1)],
                                         ones_col2, start=True, stop=True)
                    ts_sb = smt.tile([128, NQT, 2], f32, tag="tssb")
                    nc.scalar.copy(ts_sb, ts_ps[:].rearrange(
                        "q (t two) -> q t two", two=2))
                    vs = smt.tile([128, SUP], f32, tag="vs")
                    nc.vector.memset(vs, 0.0)
                    for I in range(SUP - 1):
                        nc.vector.reduce_sum(vs[:, I:I + 1],
                                             ts_sb[:, 4 * (I + 1):NQT, 0],
                                             axis=mybir.AxisListType.X)
                    tr_ps = pst2.tile([SUP, 128], f32, tag="t2")
                    nc.tensor.transpose(tr_ps, vs, eye_f)
                    sfx_sb = smt.tile([SUP, 128], f32r, tag="sfxsb")
                    nc.scalar.copy(sfx_sb, tr_ps)
                    for I in range(SUP - 1):
                        nc.sync.dma_start(out=sfxT[0:1, p, I, :],
                                          in_=sfx_sb[I:I + 1, :])

        # ======================= Phase B: attention ==========================
        with ExitStack() as ctxB:
            expool = ctxB.enter_context(tc.tile_pool(name="exp", bufs=4))
            ppool = ctxB.enter_context(tc.tile_pool(name="p", bufs=5))
            pfpool = ctxB.enter_context(tc.tile_pool(name="pf", bufs=2))
            ptpool = ctxB.enter_context(tc.tile_pool(name="pt", bufs=3))
            osup = ctxB.enter_context(tc.tile_pool(name="osup", bufs=1))
            hidp = ctxB.enter_context(tc.tile_pool(name="hid", bufs=1))
            smallp = ctxB.enter_context(tc.tile_pool(name="small", bufs=12))
            otpool = ctxB.enter_context(tc.tile_pool(name="ot", bufs=2))
            psS = ctxB.enter_context(tc.tile_pool(name="psS", bufs=2,
                                                  space="PSUM"))
            psT = ctxB.enter_context(tc.tile_pool(name="psT", bufs=2,
                                                  space="PSUM"))
            psO = ctxB.enter_context(tc.tile_pool(name="psO", bufs=2,
                                                  space="PSUM"))
            psOP = ctxB.enter_context(tc.tile_pool(name="psOP", bufs=2,
                                                   space="PSUM"))

            def emit_pf(i):
                # pad-row fill of the masked-out tail, broadcast to all heads
                Ni = 128 * (i + 1)
                Wd = S - Ni
                if Wd == 0:
                    return
                Wp = min(Wd, 1024)
                pf = pfpool.tile([128, Wp], f32, tag="pf", name=f"pf{i}")
                blend_eng.tensor_copy(pf, _bcast(padh_sb[:, i:i + 1], Wp))
                c0 = 0
                while c0 < Wd:
                    cw = min(Wp, Wd - c0)
                    dst = bass.AP(
                        tensor=prob, offset=128 * i * S + Ni + c0,
                        ap=[[S, 128], [S * S, HPC], [1, cw]])
                    pfap = pf[:]
                    src = bass.AP(
                        tensor=pfap.tensor, offset=pfap.offset,
                        ap=[pfap.ap[0], [0, HPC], [1, cw]])
                    nc.sync.dma_start(out=dst, in_=src)
                    c0 += cw

            for I in range(SUP):
                o_sup = osup.tile([128, PAIRS, 512], f32r, tag="osup")
                for h in range(HPC):
                    if h < 4:
                        emit_pf(4 * I + h)
                    pr, ro = h // 2, 64 * (h % 2)
                    p_list = []
                    for m in range(4):
                        i = 4 * I + m
                        Ni = 128 * (i + 1)
                        p_t = ppool.tile([128, S], f32r, tag="p")
                        exps = []
                        st = None
                        for c in range(i // 4 + 1):
                            base = 512 * c
                            cw = min(512, Ni - base)
                            sps = psS.tile([128, cw], f32, tag="s")
                            nc.tensor.matmul(
                                sps,
                                qhT_c[i // 4][ro:ro + 64, pr,
                                              128 * (i % 4):128 * (i % 4) + 128],
                                khT_c[c][ro:ro + 64, pr, 0:cw],
                                start=True, stop=True)
                            if base + cw == Ni:
                                nc.vector.tensor_add(
                                    sps[:, cw - 128:cw],
                                    sps[:, cw - 128:cw], tri_sb)
                            ex = expool.tile([128, cw], f32, tag="e")
                            sm = smallp.tile([128, 1], f32, tag="sm")
                            nc.scalar.activation(ex, sps, AF.Exp, scale=0.125,
                                                 accum_out=sm)
                            exps.append((ex, base, cw))
                            if st is None:
                                st = sm
                            else:
                                st2 = smallp.tile([128, 1], f32, tag="st")
                                nc.vector.tensor_add(st2, st, sm)
                                st = st2
                        rec = smallp.tile([128, 1], f32, tag="rec")
                        nc.vector.reciprocal(rec, st)
                        scl = smallp.tile([128, 1], f32, tag="scl")
                        nc.vector.tensor_mul(scl, rec, keep_sb[:, i:i + 1])
                        for (ex, b0, cw) in exps:
                            blend_eng.tensor_scalar(
                                out=p_t[:, b0:b0 + cw], in0=ex,
                                scalar1=scl, scalar2=padh_sb[:, i:i + 1],
                                op0=ALU.mult, op1=ALU.add)
                        nc.sync.dma_start(
                            out=prob[h, 128 * i:128 * (i + 1), 0:Ni],
                            in_=p_t[:, 0:Ni].bitcast(f32))
                        p_list.append(p_t)

                    # transpose P blocks, PV matmuls
                    o_ps = psO.tile([64, 512], f32, tag="o")
                    last_j = 4 * I + 3
                    for j in range(last_j + 1):
                        tps = psT.tile([128, 512], f32r, tag="t")
                        for m in range(4):
                            i = 4 * I + m
                            if j <= i:
                                nc.tensor.transpose(
                                    tps[:, 128 * m:128 * (m + 1)],
                                    p_list[m][:, 128 * j:128 * (j + 1)],
                                    eye_r)
                            else:
                                nc.tensor.matmul(
                                    tps[:, 128 * m:128 * (m + 1)].bitcast(f32),
                                    ones_row,
                                    padrow_sb[0:1, 128 * i:128 * (i + 1)],
                                    start=True, stop=True)
                        pt = ptpool.tile([128, 512], f32r, tag="pt")
                        if j % 4 == 3:
                            nc.scalar.copy(pt, tps)
                        else:
                            nc.vector.tensor_copy(pt, tps)
                        nc.tensor.matmul(
                            o_ps,
                            vhn_c[j // 4][:, j % 4,
                                          128 * pr + ro:128 * pr + ro + 64],
                            pt, start=(j == 0),
                            stop=(j == last_j and I == SUP - 1))
                    if I < SUP - 1:
                        nc.tensor.matmul(
                            o_ps, sfxT[0:1, pr, I, ro:ro + 64],
                            padrow_sb[0:1, 512 * I:512 * (I + 1)],
                            start=False, stop=True)
                    # place O^T block; odd heads need a cross-partition DMA
                    if h % 2 == 0:
                        nc.vector.tensor_copy(o_sup[0:64, pr, :], o_ps)
                    else:
                        otmp = otpool.tile([64, 512], f32r, tag="otmp")
                        nc.vector.tensor_copy(otmp, o_ps)
                        nc.sync.dma_start(out=o_sup[64:128, pr, :], in_=otmp)

                # output projection for this q super-tile
                for qm in range(4):
                    hid_t = hidp.tile([128, D], f32, tag="hid")
                    for n2 in range(D // 512):
                        op_ps = psOP.tile([128, 512], f32, tag="op")
                        for dk in range(DCOL // 128):
                            nc.tensor.matmul(
                                op_ps,
                                o_sup[:, dk, 128 * qm:128 * (qm + 1)],
                                wo_sb[:, dk, 512 * n2:512 * (n2 + 1)],
                                start=(dk == 0), stop=(dk == DCOL // 128 - 1))
                        nc.vector.tensor_copy(
                            hid_t[:, 512 * n2:512 * (n2 + 1)], op_ps)
                    r0 = 512 * I + 128 * qm
                    nc.sync.dma_start(out=hid[r0:r0 + 128, :], in_=hid_t)


# ---------------------------------------------------------------------------
# Host side
# ---------------------------------------------------------------------------

_NC_CACHE = {}


def _get_nc(S=S_FULL, HPC=HPC_FULL):
    key = (S, HPC)
    if key not in _NC_CACHE:
        _NC_CACHE[key] = build_nc(S, HPC)
    return _NC_CACHE[key]


def make_in_maps(q, k, v, mask, Wq, bq, Wk, bk, Wv, bv, Wo, bo,
                 S=S_FULL, HPC=HPC_FULL):
    """Build per-core input maps. q/k/v: [B,S,D]; mask: [B,S] bool."""
    NQT = S // 128
    DCOL = HPC * DH
    tri = np.triu(np.full((128, 128), -1e30, np.float32), 1)
    eye = np.eye(128, dtype=np.float32)
    ones = np.ones((128, 128), np.float32)
    in_maps = []
    for c in range(N_CORES):
        b, g = c // GROUPS, c % GROUPS
        sl = slice(DCOL * g, DCOL * (g + 1))
        keep = 1.0 - mask[b].astype(np.float32)
        padv = mask[b].astype(np.float32) / np.float32(S)
        in_maps.append({
            "xq": np.ascontiguousarray(q[b]),
            "xk": np.ascontiguousarray(k[b]),
            "xv": np.ascontiguousarray(v[b]),
            "wq": np.ascontiguousarray(Wq[:, sl]),
            "wk": np.ascontiguousarray(Wk[:, sl]),
            "wv": np.ascontiguousarray(Wv[:, sl]),
            "wo": np.ascontiguousarray(Wo[sl, :]),
            "b2q": np.ascontiguousarray(bq[sl].reshape(-1, 128).T),
            "b2k": np.ascontiguousarray(bk[sl].reshape(-1, 128).T),
            "b2v": np.ascontiguousarray(bv[sl].reshape(-1, 128).T),
            "keep": np.ascontiguousarray(keep.reshape(NQT, 128).T),
            "padh": np.ascontiguousarray(padv.reshape(NQT, 128).T),
            "padrow": np.ascontiguousarray(padv[None, :]),
            "tri": tri,
            "eye": eye,
            "ones_in": ones,
        })
    return in_maps


def assemble(results, bo, S=S_FULL, HPC=HPC_FULL):
    prob = np.empty((B, HEADS, S, S), np.float32)
    hidden = np.empty((B, S, D), np.float32)
    for c in range(N_CORES):
        b, g = c // GROUPS, c % GROUPS
        prob[b, HPC * g:HPC * (g + 1)] = results[c]["prob"]
    for b in range(B):
        hidden[b] = results[GROUPS * b]["hid"]
        for g in range(1, GROUPS):
            hidden[b] += results[GROUPS * b + g]["hid"]
        hidden[b] += bo
    return hidden, prob


def kernel(q, k, v, mask, Wq, bq, Wk, bk, Wv, bv, Wo, bo, reverse):
    q = np.asarray(q, np.float32)
    k = np.asarray(k, np.float32)
    v = np.asarray(v, np.float32)
    mask = np.asarray(mask, bool)
    Wq, bq = np.asarray(Wq, np.float32), np.asarray(bq, np.float32)
    Wk, bk = np.asarray(Wk, np.float32), np.asarray(bk, np.float32)
    Wv, bv = np.asarray(Wv, np.float32), np.asarray(bv, np.float32)
    Wo, bo = np.asarray(Wo, np.float32), np.asarray(bo, np.float32)
    rev = int(np.asarray(reverse))

    if rev:
        q, k, v, mask = (q[:, ::-1], k[:, ::-1], v[:, ::-1], mask[:, ::-1])

    nc = _get_nc()
    in_maps = make_in_maps(q, k, v, mask, Wq, bq, Wk, bk, Wv, bv, Wo, bo)
    res = run_bass_kernel_spmd(nc, in_maps, core_ids=list(range(N_CORES)))
    hidden, prob = assemble(res.results, bo)

    if rev:
        hidden = np.ascontiguousarray(hidden[:, ::-1])
        prob = np.ascontiguousarray(prob[:, :, ::-1, ::-1])
    return hidden, prob
